# revision 11
# baseline (speedup 1.0000x reference)
"""Trainium2 Bass kernel for attention-based memory retrieval (retrieval_knn).

Problem: query [B=2048, D=1024] against memory bank [M=65536, D=1024]:
  attn = softmax(Q K^T / sqrt(D)); mask attn > 2e-5; renormalize;
  top-64 per query; gather memory_keys/values rows; return
  (retrieved_keys [B,64,D], retrieved_values [B,64,D], top_weights [B,64]).

Strategy (8 NeuronCores):
  Launch A (query-sharded, 256 queries/core, ~1.51 ms): stream full K^T
    once; fp32-accurate matmul as 3 full-rate fp16 hi/lo matmuls
    (qh*kh + qh*kl + ql*kh, dropped ql*kl term ~2^-24) -> ACT exp with
    fused row-sum accumulation (Z, unshifted exp is safe: |logit| < 8)
    -> per-512-block top-8 candidates (DVE max/max_index) -> spill E to
    HBM scratch; pass 2 re-reads E and computes the exact masked softmax
    sum SM = sum_{E > 2e-5 * Z} E via ACT relu-accum + DVE count-accum.
  Host: merge 1024 candidates/row -> exact global top-64 (value-sorted,
    jax tie semantics), weights = E / (SM + 1e-8 * Z); rare-row fallback
    (block overflow / ties / sub-threshold winners) recomputed exactly.
  Launch B (bank-sharded deduplicated gather, ~0.35 ms): of B*64 = 131072
    winner selections only ~57k bank rows are distinct; each core owns
    M/8 = 8192 bank rows and gathers the distinct winners in its shard
    via indirect DMA; the host replicates rows to output positions.

Everything here is self-contained: only numpy/jax + the system concourse
(bass) toolchain are used. Shapes/sharding are hardcoded for the problem.
"""
import sys

if "/opt/trn_rl_repo" not in sys.path:
    sys.path.insert(0, "/opt/trn_rl_repo")

import math
import numpy as np

import concourse.bass as bass
import concourse.bacc as bacc
import concourse.tile as tile
import concourse.mybir as mybir

# ----------------------------------------------------------------------------
# Problem constants
# ----------------------------------------------------------------------------
B = 2048
M = 65536
D = 1024
TOP_K = 64
THRESHOLD = 2e-5
N_CORES = 8

P = 128                 # partitions / q-tile rows
BLK = 512               # m-block width (candidate block)
DCH = D // 128          # 8 contraction chunks
SCALE = 1.0 / math.sqrt(D)

NQ = B // N_CORES       # 256 queries per core
NQT = NQ // P           # 2 q-tiles per core
NB = M // BLK           # 128 m-blocks
NCAND = NB * 8          # 1024 candidates per row

P2CH = 4096             # pass-2 chunk width
NP2 = M // P2CH         # 16 pass-2 chunks

MM_MODE = "fp16x3"     # "fp32" (1/4-rate PE) or "fp16x3" hi/lo split (3/4... full-rate)

MSH = M // N_CORES      # 8192 bank rows per core (launch B)
GCAP = 7424             # max UNIQUE gathered rows per shard (58*128); ~7085 expected
GT = GCAP // P          # 58 gather tiles


# ----------------------------------------------------------------------------
# Launch A: scores + stats + candidates
# ----------------------------------------------------------------------------
def build_launch_a():
    nc = bacc.Bacc(None, target_bir_lowering=False)
    f32 = mybir.dt.float32
    f16 = mybir.dt.float16
    if MM_MODE == "fp32":
        qt_d = nc.dram_tensor("qt", [D, NQ], f32, kind="ExternalInput")
        kt_d = nc.dram_tensor("kt", [D, M], f32, kind="ExternalInput")
    else:
        qth_d = nc.dram_tensor("qth", [D, NQ], f16, kind="ExternalInput")
        qtl_d = nc.dram_tensor("qtl", [D, NQ], f16, kind="ExternalInput")
        kth_d = nc.dram_tensor("kth", [D, M], f16, kind="ExternalInput")
        ktl_d = nc.dram_tensor("ktl", [D, M], f16, kind="ExternalInput")
    cv_d = nc.dram_tensor("cand_val", [NQ, NCAND], f32, kind="ExternalOutput")
    ci_d = nc.dram_tensor("cand_idx", [NQ, NCAND], mybir.dt.uint16, kind="ExternalOutput")
    z_d = nc.dram_tensor("zsum", [NQ, 1], f32, kind="ExternalOutput")
    sm_d = nc.dram_tensor("smsum", [NQ, 1], f32, kind="ExternalOutput")
    tau_d = nc.dram_tensor("tau", [NQ, 1], f32, kind="ExternalOutput")
    espill = nc.dram_tensor("espill", [NQ, M], f32)  # internal scratch

    with tile.TileContext(nc) as tc:
        with (
            tc.tile_pool(name="ktp", bufs=3) as ktp,
            tc.tile_pool(name="qp", bufs=1) as qp,
            tc.tile_pool(name="ep", bufs=3) as ep,
            tc.tile_pool(name="psum", bufs=4, space="PSUM") as psp,
            tc.tile_pool(name="stat", bufs=1) as stat,
            tc.tile_pool(name="p2", bufs=2) as p2,
        ):
            if MM_MODE == "fp32":
                qt_sb = qp.tile([128, DCH, NQ], f32)
                nc.sync.dma_start(out=qt_sb[:], in_=qt_d.rearrange("(c p) q -> p c q", p=128))
            else:
                qth_sb = qp.tile([128, DCH, NQ], f16)
                qtl_sb = qp.tile([128, DCH, NQ], f16)
                nc.sync.dma_start(out=qth_sb[:], in_=qth_d.rearrange("(c p) q -> p c q", p=128))
                nc.sync.dma_start(out=qtl_sb[:], in_=qtl_d.rearrange("(c p) q -> p c q", p=128))

            zcols = [stat.tile([P, NB], f32, tag=f"zcols{i}", name=f"zcols{i}") for i in range(NQT)]
            cval = [stat.tile([P, NCAND], f32, tag=f"cval{i}", name=f"cval{i}") for i in range(NQT)]
            cidx = [stat.tile([P, NCAND], mybir.dt.uint16, tag=f"cidx{i}", name=f"cidx{i}") for i in range(NQT)]

            # ---- pass 1: matmul -> exp(+Z accum) -> top8/block -> spill ----
            for b in range(NB):
                if MM_MODE == "fp32":
                    kt_sb = ktp.tile([128, DCH, BLK], f32, tag="kt")
                    nc.sync.dma_start(
                        out=kt_sb[:],
                        in_=kt_d[:, b * BLK:(b + 1) * BLK].rearrange("(c p) m -> p c m", p=128),
                    )
                else:
                    kth_sb = ktp.tile([128, DCH, BLK], f16, tag="kth")
                    ktl_sb = ktp.tile([128, DCH, BLK], f16, tag="ktl")
                    nc.sync.dma_start(
                        out=kth_sb[:],
                        in_=kth_d[:, b * BLK:(b + 1) * BLK].rearrange("(c p) m -> p c m", p=128),
                    )
                    nc.sync.dma_start(
                        out=ktl_sb[:],
                        in_=ktl_d[:, b * BLK:(b + 1) * BLK].rearrange("(c p) m -> p c m", p=128),
                    )
                for qt in range(NQT):
                    ps = psp.tile([P, BLK], f32, tag="ps")
                    if MM_MODE == "fp32":
                        for c in range(DCH):
                            nc.tensor.matmul(
                                ps[:],
                                lhsT=qt_sb[:, c, qt * P:(qt + 1) * P],
                                rhs=kt_sb[:, c, :],
                                start=(c == 0),
                                stop=(c == DCH - 1),
                            )
                    else:
                        qsl = slice(qt * P, (qt + 1) * P)
                        n_mm = DCH * 3
                        i_mm = 0
                        for c in range(DCH):
                            for lhs_t, rhs_t in (
                                (qth_sb, kth_sb), (qth_sb, ktl_sb), (qtl_sb, kth_sb),
                            ):
                                nc.tensor.matmul(
                                    ps[:],
                                    lhsT=lhs_t[:, c, qsl],
                                    rhs=rhs_t[:, c, :],
                                    start=(i_mm == 0),
                                    stop=(i_mm == n_mm - 1),
                                )
                                i_mm += 1
                    e_sb = ep.tile([P, BLK], f32, tag="e")
                    nc.scalar.activation(
                        out=e_sb[:],
                        in_=ps[:],
                        func=mybir.ActivationFunctionType.Exp,
                        scale=SCALE,
                        accum_out=zcols[qt][:, b:b + 1],
                    )
                    nc.vector.max(out=cval[qt][:, b * 8:(b + 1) * 8], in_=e_sb[:])
                    nc.vector.max_index(
                        out=cidx[qt][:, b * 8:(b + 1) * 8],
                        in_max=cval[qt][:, b * 8:(b + 1) * 8],
                        in_values=e_sb[:],
                    )
                    nc.sync.dma_start(
                        out=espill[qt * P:(qt + 1) * P, b * BLK:(b + 1) * BLK],
                        in_=e_sb[:],
                    )

            # ---- stats: Z, tau ----
            zred = [stat.tile([P, 1], f32, tag=f"zred{i}", name=f"zred{i}") for i in range(NQT)]
            tau = [stat.tile([P, 1], f32, tag=f"tau{i}", name=f"tau{i}") for i in range(NQT)]
            ntau = [stat.tile([P, 1], f32, tag=f"ntau{i}", name=f"ntau{i}") for i in range(NQT)]
            for qt in range(NQT):
                nc.vector.reduce_sum(zred[qt][:], zcols[qt][:], axis=mybir.AxisListType.X)
                nc.vector.tensor_scalar_mul(tau[qt][:], zred[qt][:], float(THRESHOLD))
                nc.vector.tensor_scalar_mul(ntau[qt][:], tau[qt][:], -1.0)

            # ---- pass 2: SM = sum_{E > tau} E  (relu-sum on ACT, count on DVE) ----
            srcols = [stat.tile([P, NP2], f32, tag=f"srcols{i}", name=f"srcols{i}") for i in range(NQT)]
            cncols = [stat.tile([P, NP2], f32, tag=f"cncols{i}", name=f"cncols{i}") for i in range(NQT)]
            for qt in range(NQT):
                for i in range(NP2):
                    ech = p2.tile([P, P2CH], f32, tag="ech")
                    nc.sync.dma_start(
                        out=ech[:],
                        in_=espill[qt * P:(qt + 1) * P, i * P2CH:(i + 1) * P2CH],
                    )
                    rj = p2.tile([P, P2CH], f32, tag="rj")
                    nc.scalar.activation(
                        out=rj[:],
                        in_=ech[:],
                        func=mybir.ActivationFunctionType.Relu,
                        bias=ntau[qt][:, 0:1],
                        accum_out=srcols[qt][:, i:i + 1],
                    )
                    cj = p2.tile([P, P2CH], f32, tag="cj")
                    nc.vector.tensor_scalar(
                        cj[:],
                        ech[:],
                        tau[qt][:, 0:1],
                        0.0,
                        op0=mybir.AluOpType.is_gt,
                        op1=mybir.AluOpType.add,
                        accum_out=cncols[qt][:, i:i + 1],
                    )

            # SM = rowsum(srcols) + tau * rowsum(cncols)
            for qt in range(NQT):
                sr = stat.tile([P, 1], f32, tag=f"sr{qt}")
                cn = stat.tile([P, 1], f32, tag=f"cn{qt}")
                nc.vector.reduce_sum(sr[:], srcols[qt][:], axis=mybir.AxisListType.X)
                nc.vector.reduce_sum(cn[:], cncols[qt][:], axis=mybir.AxisListType.X)
                smv = stat.tile([P, 1], f32, tag=f"smv{qt}")
                nc.vector.tensor_tensor(smv[:], cn[:], tau[qt][:], op=mybir.AluOpType.mult)
                nc.vector.tensor_tensor(smv[:], smv[:], sr[:], op=mybir.AluOpType.add)
                nc.sync.dma_start(out=z_d[qt * P:(qt + 1) * P, :], in_=zred[qt][:])
                nc.sync.dma_start(out=sm_d[qt * P:(qt + 1) * P, :], in_=smv[:])
                nc.sync.dma_start(out=tau_d[qt * P:(qt + 1) * P, :], in_=tau[qt][:])
                nc.sync.dma_start(out=cv_d[qt * P:(qt + 1) * P, :], in_=cval[qt][:])
                nc.sync.dma_start(out=ci_d[qt * P:(qt + 1) * P, :], in_=cidx[qt][:])
    nc.compile()
    return nc


# ----------------------------------------------------------------------------
# Launch B: bank-sharded winner gather
# ----------------------------------------------------------------------------
def build_launch_b():
    nc = bacc.Bacc(None, target_bir_lowering=False)
    f32 = mybir.dt.float32
    ksh = nc.dram_tensor("ksh", [MSH, D], f32, kind="ExternalInput")
    vsh = nc.dram_tensor("vsh", [MSH, D], f32, kind="ExternalInput")
    idx = nc.dram_tensor("idx", [GCAP, 1], mybir.dt.int32, kind="ExternalInput")
    kg = nc.dram_tensor("kg", [GCAP, D], f32, kind="ExternalOutput")
    vg = nc.dram_tensor("vg", [GCAP, D], f32, kind="ExternalOutput")

    with tile.TileContext(nc) as tc:
        with (
            tc.tile_pool(name="gp", bufs=4) as gp,
            tc.tile_pool(name="ip", bufs=1) as ip,
        ):
            idx_sb = ip.tile([P, GT], mybir.dt.int32)
            nc.sync.dma_start(out=idx_sb[:], in_=idx.rearrange("(t p) one -> p (t one)", p=P))
            for t in range(GT):
                for src, dst, tag in ((ksh, kg, "k"), (vsh, vg, "v")):
                    g = gp.tile([P, D], f32, tag=tag)
                    nc.gpsimd.indirect_dma_start(
                        out=g[:],
                        out_offset=None,
                        in_=src[:],
                        in_offset=bass.IndirectOffsetOnAxis(ap=idx_sb[:, t:t + 1], axis=0),
                    )
                    nc.sync.dma_start(out=dst[t * P:(t + 1) * P, :], in_=g[:])
    nc.compile()
    return nc


# ----------------------------------------------------------------------------
# Kernel execution via the canonical bass_utils.run_bass_kernel_spmd
# ----------------------------------------------------------------------------
from concourse.bass_utils import run_bass_kernel_spmd

_cache = {}
LAST_HW_NS = None


def _get_module(which):
    """Build (once) and cache the compiled Bass module + cost-model estimate."""
    if which not in _cache:
        nc = build_launch_a() if which == "a" else build_launch_b()
        try:
            from concourse.timeline_sim import TimelineSim
            predicted_ns = int(TimelineSim(nc, trace=False).simulate())
        except Exception:
            predicted_ns = 0
        _cache[which] = (nc, predicted_ns)
    return _cache[which]


def _run(which, in_maps):
    nc, _ = _get_module(which)
    res = run_bass_kernel_spmd(nc, in_maps, core_ids=list(range(N_CORES)))
    return res.results


def _host_merge(cand_val, cand_idx, zsum, smsum):
    """Exact global top-64 from per-block top-8 candidates; returns
    (top_idx [B,64] int64, top_w [B,64] f32, flags [B] bool)."""
    Bq = cand_val.shape[0]
    base = (np.arange(NB, dtype=np.int64) * BLK).repeat(8)[None, :]   # [1, 1024]
    gidx = cand_idx.astype(np.int64) + base                            # [B, 1024]
    # sort by (-value, index): replicate jax.lax.top_k tie semantics
    ordv = np.argsort(-cand_val, axis=1, kind="stable")
    # stable argsort on -val keeps original (block-order = index-order within
    # equal values ONLY if candidate positions are index-sorted; positions are
    # block-major so equal values tie-break by block position == index order
    # across blocks, but within a block top-8 are value-sorted. For exact-tie
    # values inside one block max_index may even return duplicate positions;
    # rows where the winning set has duplicate or tied values are flagged and
    # recomputed exactly on the host.
    rows = np.arange(Bq)[:, None]
    sidx = gidx[rows, ordv][:, :TOP_K]
    sval = cand_val[rows, ordv][:, :TOP_K]

    # weights, reference formula: attn/(S + 1e-8) with attn = E/Z, S = SM/Z
    denom = (smsum + np.float32(1e-8) * zsum).astype(np.float32)      # [B,1]
    topw = (sval / denom).astype(np.float32)

    # --- safety flags ---
    v64 = sval[:, -1:]                                                 # [B,1]
    blk8 = cand_val.reshape(Bq, NB, 8)
    # a block whose 8th-best is >= the global 64th might hide a 9th member
    overflow = (blk8[:, :, 7] >= v64).any(axis=1)
    # duplicate winner indices (exact-tie artifacts)
    ss = np.sort(sidx, axis=1)
    dup = (ss[:, 1:] == ss[:, :-1]).any(axis=1)
    # winner below threshold (would be masked in reference)
    below = (sval <= (THRESHOLD * zsum).astype(np.float32)).any(axis=1)
    flags = overflow | dup | below
    return sidx, topw, flags


def _host_exact_row(q_row, memory_keys):
    """Full-precision fp32 reference for one row (fallback path)."""
    l = (q_row[None, :] @ memory_keys.T).astype(np.float32)[0] * np.float32(SCALE)
    l = l - l.max()
    e = np.exp(l, dtype=np.float32)
    a = e / e.sum(dtype=np.float32)
    mask = a > np.float32(THRESHOLD)
    am = np.where(mask, a, np.float32(0))
    am = am / (am.sum(dtype=np.float32) + np.float32(1e-8))
    order = np.lexsort((np.arange(M), -am))[:TOP_K]
    return order.astype(np.int64), am[order].astype(np.float32)


def kernel(query, memory_keys, memory_values, top_k=TOP_K, **_ignored):
    assert int(top_k) == TOP_K
    query = np.asarray(query, dtype=np.float32)
    memory_keys = np.asarray(memory_keys, dtype=np.float32)
    memory_values = np.asarray(memory_values, dtype=np.float32)

    # ---------------- Launch A ----------------
    qt_full = np.ascontiguousarray(query.T)          # [D, B]
    kt_full = np.ascontiguousarray(memory_keys.T)    # [D, M]
    if MM_MODE == "fp32":
        in_maps = [
            {"qt": np.ascontiguousarray(qt_full[:, c * NQ:(c + 1) * NQ]), "kt": kt_full}
            for c in range(N_CORES)
        ]
    else:
        qth = qt_full.astype(np.float16)
        qtl = (qt_full - qth.astype(np.float32)).astype(np.float16)
        kth = kt_full.astype(np.float16)
        ktl = (kt_full - kth.astype(np.float32)).astype(np.float16)
        in_maps = [
            {
                "qth": np.ascontiguousarray(qth[:, c * NQ:(c + 1) * NQ]),
                "qtl": np.ascontiguousarray(qtl[:, c * NQ:(c + 1) * NQ]),
                "kth": kth,
                "ktl": ktl,
            }
            for c in range(N_CORES)
        ]
    res_a = _run("a", in_maps)

    cand_val = np.concatenate([r["cand_val"] for r in res_a], axis=0)   # [B, 1024]
    cand_idx = np.concatenate([r["cand_idx"] for r in res_a], axis=0)
    zsum = np.concatenate([r["zsum"] for r in res_a], axis=0)           # [B, 1]
    smsum = np.concatenate([r["smsum"] for r in res_a], axis=0)

    top_idx, top_w, flags = _host_merge(cand_val, cand_idx, zsum, smsum)
    for r in np.nonzero(flags)[0]:
        top_idx[r], top_w[r] = _host_exact_row(query[r], memory_keys)

    # ---------------- Launch B (deduplicated gather) ----------------
    flat_idx = top_idx.reshape(-1)                   # [B*64]
    uniq, inverse = np.unique(flat_idx, return_inverse=True)
    shard_of_u = uniq // MSH
    in_maps_b = []
    sel_lists = []
    for s in range(N_CORES):
        sel = np.nonzero(shard_of_u == s)[0]
        if sel.size > GCAP:
            sel = sel[:GCAP]  # excess unique rows fall back to the host fill
        loc = np.zeros((GCAP, 1), dtype=np.int32)
        loc[:sel.size, 0] = (uniq[sel] - s * MSH).astype(np.int32)
        sel_lists.append(sel)
        in_maps_b.append({
            "ksh": memory_keys[s * MSH:(s + 1) * MSH],
            "vsh": memory_values[s * MSH:(s + 1) * MSH],
            "idx": loc,
        })
    res_b = _run("b", in_maps_b)

    ktab = np.empty((uniq.size, D), dtype=np.float32)
    vtab = np.empty((uniq.size, D), dtype=np.float32)
    covered = np.zeros(uniq.size, dtype=bool)
    for s in range(N_CORES):
        sel = sel_lists[s]
        ktab[sel] = res_b[s]["kg"][:sel.size]
        vtab[sel] = res_b[s]["vg"][:sel.size]
        covered[sel] = True
    if not covered.all():
        rest = np.nonzero(~covered)[0]
        ktab[rest] = memory_keys[uniq[rest]]
        vtab[rest] = memory_values[uniq[rest]]
    retrieved_keys = ktab[inverse].reshape(B, TOP_K, D)
    retrieved_values = vtab[inverse].reshape(B, TOP_K, D)

    global LAST_HW_NS
    LAST_HW_NS = _get_module("a")[1] + _get_module("b")[1]
    return retrieved_keys, retrieved_values, top_w


# revision 13
# speedup vs baseline: 1.0373x; 1.0373x over previous
"""Trainium2 Bass kernel for attention-based memory retrieval (retrieval_knn).

Problem: query [B=2048, D=1024] against memory bank [M=65536, D=1024]:
  attn = softmax(Q K^T / sqrt(D)); mask attn > 2e-5; renormalize;
  top-64 per query; gather memory_keys/values rows; return
  (retrieved_keys [B,64,D], retrieved_values [B,64,D], top_weights [B,64]).

Strategy (8 NeuronCores):
  Launch A (query-sharded, 256 queries/core, ~1.51 ms): stream full K^T
    once; fp32-accurate matmul as 3 full-rate fp16 hi/lo matmuls
    (qh*kh + qh*kl + ql*kh, dropped ql*kl term ~2^-24) -> ACT exp with
    fused row-sum accumulation (Z, unshifted exp is safe: |logit| < 8)
    -> per-512-block top-8 candidates (DVE max/max_index) -> spill E to
    HBM scratch; pass 2 re-reads E and computes the exact masked softmax
    sum SM = sum_{E > 2e-5 * Z} E via ACT relu-accum + DVE count-accum.
  Host: merge 1024 candidates/row -> exact global top-64 (value-sorted,
    jax tie semantics), weights = E / (SM + 1e-8 * Z); rare-row fallback
    (block overflow / ties / sub-threshold winners) recomputed exactly.
  Launch B (bank-sharded deduplicated gather, ~0.35 ms): of B*64 = 131072
    winner selections only ~57k bank rows are distinct; each core owns
    M/8 = 8192 bank rows and gathers the distinct winners in its shard
    via indirect DMA; the host replicates rows to output positions.

Everything here is self-contained: only numpy/jax + the system concourse
(bass) toolchain are used. Shapes/sharding are hardcoded for the problem.
"""
import sys

if "/opt/trn_rl_repo" not in sys.path:
    sys.path.insert(0, "/opt/trn_rl_repo")

import math
import numpy as np

import concourse.bass as bass
import concourse.bacc as bacc
import concourse.tile as tile
import concourse.mybir as mybir

# ----------------------------------------------------------------------------
# Problem constants
# ----------------------------------------------------------------------------
B = 2048
M = 65536
D = 1024
TOP_K = 64
THRESHOLD = 2e-5
N_CORES = 8

P = 128                 # partitions / q-tile rows
BLK = 512               # m-block width (candidate block)
DCH = D // 128          # 8 contraction chunks
SCALE = 1.0 / math.sqrt(D)

NQ = B // N_CORES       # 256 queries per core
NQT = NQ // P           # 2 q-tiles per core
NB = M // BLK           # 128 m-blocks
NCAND = NB * 8          # 1024 candidates per row

P2CH = 4096             # pass-2 chunk width
NP2 = M // P2CH         # 16 pass-2 chunks

MM_MODE = "fp16x3"     # "fp32" (1/4-rate PE) or "fp16x3" hi/lo split (3/4... full-rate)

MSH = M // N_CORES      # 8192 bank rows per core (launch B)
GCAP = 7424             # max UNIQUE gathered rows per shard (58*128); ~7085 expected
GT = GCAP // P          # 58 gather tiles


# ----------------------------------------------------------------------------
# Launch A: scores + stats + candidates
# ----------------------------------------------------------------------------
def build_launch_a():
    nc = bacc.Bacc(None, target_bir_lowering=False)
    f32 = mybir.dt.float32
    f16 = mybir.dt.float16
    if MM_MODE == "fp32":
        qt_d = nc.dram_tensor("qt", [D, NQ], f32, kind="ExternalInput")
        kt_d = nc.dram_tensor("kt", [D, M], f32, kind="ExternalInput")
    else:
        qth_d = nc.dram_tensor("qth", [D, NQ], f16, kind="ExternalInput")
        qtl_d = nc.dram_tensor("qtl", [D, NQ], f16, kind="ExternalInput")
        kth_d = nc.dram_tensor("kth", [D, M], f16, kind="ExternalInput")
        ktl_d = nc.dram_tensor("ktl", [D, M], f16, kind="ExternalInput")
    cv_d = nc.dram_tensor("cand_val", [NQ, NCAND], f32, kind="ExternalOutput")
    ci_d = nc.dram_tensor("cand_idx", [NQ, NCAND], mybir.dt.uint16, kind="ExternalOutput")
    z_d = nc.dram_tensor("zsum", [NQ, 1], f32, kind="ExternalOutput")
    sm_d = nc.dram_tensor("smsum", [NQ, 1], f32, kind="ExternalOutput")
    tau_d = nc.dram_tensor("tau", [NQ, 1], f32, kind="ExternalOutput")
    espill = nc.dram_tensor("espill", [NQ, M], f16)  # internal scratch (fp16: only feeds the masked-sum threshold pass)

    with tile.TileContext(nc) as tc:
        with (
            tc.tile_pool(name="ktp", bufs=3) as ktp,
            tc.tile_pool(name="qp", bufs=1) as qp,
            tc.tile_pool(name="ep", bufs=3) as ep,
            tc.tile_pool(name="psum", bufs=4, space="PSUM") as psp,
            tc.tile_pool(name="stat", bufs=1) as stat,
            tc.tile_pool(name="p2", bufs=2) as p2,
        ):
            if MM_MODE == "fp32":
                qt_sb = qp.tile([128, DCH, NQ], f32)
                nc.sync.dma_start(out=qt_sb[:], in_=qt_d.rearrange("(c p) q -> p c q", p=128))
            else:
                qth_sb = qp.tile([128, DCH, NQ], f16)
                qtl_sb = qp.tile([128, DCH, NQ], f16)
                nc.sync.dma_start(out=qth_sb[:], in_=qth_d.rearrange("(c p) q -> p c q", p=128))
                nc.sync.dma_start(out=qtl_sb[:], in_=qtl_d.rearrange("(c p) q -> p c q", p=128))

            zcols = [stat.tile([P, NB], f32, tag=f"zcols{i}", name=f"zcols{i}") for i in range(NQT)]
            cval = [stat.tile([P, NCAND], f32, tag=f"cval{i}", name=f"cval{i}") for i in range(NQT)]
            cidx = [stat.tile([P, NCAND], mybir.dt.uint16, tag=f"cidx{i}", name=f"cidx{i}") for i in range(NQT)]

            # ---- pass 1: matmul -> exp(+Z accum) -> top8/block -> spill ----
            for b in range(NB):
                if MM_MODE == "fp32":
                    kt_sb = ktp.tile([128, DCH, BLK], f32, tag="kt")
                    nc.sync.dma_start(
                        out=kt_sb[:],
                        in_=kt_d[:, b * BLK:(b + 1) * BLK].rearrange("(c p) m -> p c m", p=128),
                    )
                else:
                    kth_sb = ktp.tile([128, DCH, BLK], f16, tag="kth")
                    ktl_sb = ktp.tile([128, DCH, BLK], f16, tag="ktl")
                    nc.sync.dma_start(
                        out=kth_sb[:],
                        in_=kth_d[:, b * BLK:(b + 1) * BLK].rearrange("(c p) m -> p c m", p=128),
                    )
                    nc.sync.dma_start(
                        out=ktl_sb[:],
                        in_=ktl_d[:, b * BLK:(b + 1) * BLK].rearrange("(c p) m -> p c m", p=128),
                    )
                for qt in range(NQT):
                    ps = psp.tile([P, BLK], f32, tag="ps")
                    if MM_MODE == "fp32":
                        for c in range(DCH):
                            nc.tensor.matmul(
                                ps[:],
                                lhsT=qt_sb[:, c, qt * P:(qt + 1) * P],
                                rhs=kt_sb[:, c, :],
                                start=(c == 0),
                                stop=(c == DCH - 1),
                            )
                    else:
                        qsl = slice(qt * P, (qt + 1) * P)
                        n_mm = DCH * 3
                        i_mm = 0
                        for c in range(DCH):
                            for lhs_t, rhs_t in (
                                (qth_sb, kth_sb), (qth_sb, ktl_sb), (qtl_sb, kth_sb),
                            ):
                                nc.tensor.matmul(
                                    ps[:],
                                    lhsT=lhs_t[:, c, qsl],
                                    rhs=rhs_t[:, c, :],
                                    start=(i_mm == 0),
                                    stop=(i_mm == n_mm - 1),
                                )
                                i_mm += 1
                    e_sb = ep.tile([P, BLK], f32, tag="e")
                    nc.scalar.activation(
                        out=e_sb[:],
                        in_=ps[:],
                        func=mybir.ActivationFunctionType.Exp,
                        scale=SCALE,
                        accum_out=zcols[qt][:, b:b + 1],
                    )
                    nc.vector.max(out=cval[qt][:, b * 8:(b + 1) * 8], in_=e_sb[:])
                    nc.vector.max_index(
                        out=cidx[qt][:, b * 8:(b + 1) * 8],
                        in_max=cval[qt][:, b * 8:(b + 1) * 8],
                        in_values=e_sb[:],
                    )
                    nc.gpsimd.dma_start(
                        out=espill[qt * P:(qt + 1) * P, b * BLK:(b + 1) * BLK],
                        in_=e_sb[:],
                    )

            # ---- stats: Z, tau ----
            zred = [stat.tile([P, 1], f32, tag=f"zred{i}", name=f"zred{i}") for i in range(NQT)]
            tau = [stat.tile([P, 1], f32, tag=f"tau{i}", name=f"tau{i}") for i in range(NQT)]
            ntau = [stat.tile([P, 1], f32, tag=f"ntau{i}", name=f"ntau{i}") for i in range(NQT)]
            for qt in range(NQT):
                nc.vector.reduce_sum(zred[qt][:], zcols[qt][:], axis=mybir.AxisListType.X)
                nc.vector.tensor_scalar_mul(tau[qt][:], zred[qt][:], float(THRESHOLD))
                nc.vector.tensor_scalar_mul(ntau[qt][:], tau[qt][:], -1.0)

            # ---- pass 2: SM = sum_{E > tau} E  (relu-sum on ACT, count on DVE) ----
            srcols = [stat.tile([P, NP2], f32, tag=f"srcols{i}", name=f"srcols{i}") for i in range(NQT)]
            cncols = [stat.tile([P, NP2], f32, tag=f"cncols{i}", name=f"cncols{i}") for i in range(NQT)]
            for qt in range(NQT):
                for i in range(NP2):
                    ech = p2.tile([P, P2CH], f16, tag="ech", bufs=4)
                    nc.sync.dma_start(
                        out=ech[:],
                        in_=espill[qt * P:(qt + 1) * P, i * P2CH:(i + 1) * P2CH],
                    )
                    rj = p2.tile([P, P2CH], f16, tag="rj")
                    nc.scalar.activation(
                        out=rj[:],
                        in_=ech[:],
                        func=mybir.ActivationFunctionType.Relu,
                        bias=ntau[qt][:, 0:1],
                        accum_out=srcols[qt][:, i:i + 1],
                    )
                    cj = p2.tile([P, P2CH], f16, tag="cj")
                    nc.vector.tensor_scalar(
                        cj[:],
                        ech[:],
                        tau[qt][:, 0:1],
                        0.0,
                        op0=mybir.AluOpType.is_gt,
                        op1=mybir.AluOpType.add,
                        accum_out=cncols[qt][:, i:i + 1],
                    )

            # SM = rowsum(srcols) + tau * rowsum(cncols)
            for qt in range(NQT):
                sr = stat.tile([P, 1], f32, tag=f"sr{qt}")
                cn = stat.tile([P, 1], f32, tag=f"cn{qt}")
                nc.vector.reduce_sum(sr[:], srcols[qt][:], axis=mybir.AxisListType.X)
                nc.vector.reduce_sum(cn[:], cncols[qt][:], axis=mybir.AxisListType.X)
                smv = stat.tile([P, 1], f32, tag=f"smv{qt}")
                nc.vector.tensor_tensor(smv[:], cn[:], tau[qt][:], op=mybir.AluOpType.mult)
                nc.vector.tensor_tensor(smv[:], smv[:], sr[:], op=mybir.AluOpType.add)
                nc.sync.dma_start(out=z_d[qt * P:(qt + 1) * P, :], in_=zred[qt][:])
                nc.sync.dma_start(out=sm_d[qt * P:(qt + 1) * P, :], in_=smv[:])
                nc.sync.dma_start(out=tau_d[qt * P:(qt + 1) * P, :], in_=tau[qt][:])
                nc.sync.dma_start(out=cv_d[qt * P:(qt + 1) * P, :], in_=cval[qt][:])
                nc.sync.dma_start(out=ci_d[qt * P:(qt + 1) * P, :], in_=cidx[qt][:])
    nc.compile()
    return nc


# ----------------------------------------------------------------------------
# Launch B: bank-sharded winner gather
# ----------------------------------------------------------------------------
def build_launch_b():
    nc = bacc.Bacc(None, target_bir_lowering=False)
    f32 = mybir.dt.float32
    ksh = nc.dram_tensor("ksh", [MSH, D], f32, kind="ExternalInput")
    vsh = nc.dram_tensor("vsh", [MSH, D], f32, kind="ExternalInput")
    idx = nc.dram_tensor("idx", [GCAP, 1], mybir.dt.int32, kind="ExternalInput")
    kg = nc.dram_tensor("kg", [GCAP, D], f32, kind="ExternalOutput")
    vg = nc.dram_tensor("vg", [GCAP, D], f32, kind="ExternalOutput")

    with tile.TileContext(nc) as tc:
        with (
            tc.tile_pool(name="gp", bufs=4) as gp,
            tc.tile_pool(name="ip", bufs=1) as ip,
        ):
            idx_sb = ip.tile([P, GT], mybir.dt.int32)
            nc.sync.dma_start(out=idx_sb[:], in_=idx.rearrange("(t p) one -> p (t one)", p=P))
            for t in range(GT):
                for src, dst, tag in ((ksh, kg, "k"), (vsh, vg, "v")):
                    g = gp.tile([P, D], f32, tag=tag)
                    nc.gpsimd.indirect_dma_start(
                        out=g[:],
                        out_offset=None,
                        in_=src[:],
                        in_offset=bass.IndirectOffsetOnAxis(ap=idx_sb[:, t:t + 1], axis=0),
                    )
                    nc.sync.dma_start(out=dst[t * P:(t + 1) * P, :], in_=g[:])
    nc.compile()
    return nc


# ----------------------------------------------------------------------------
# Kernel execution via the canonical bass_utils.run_bass_kernel_spmd
# ----------------------------------------------------------------------------
from concourse.bass_utils import run_bass_kernel_spmd

_cache = {}
LAST_HW_NS = None


def _get_module(which):
    """Build (once) and cache the compiled Bass module + cost-model estimate."""
    if which not in _cache:
        nc = build_launch_a() if which == "a" else build_launch_b()
        try:
            from concourse.timeline_sim import TimelineSim
            predicted_ns = int(TimelineSim(nc, trace=False).simulate())
        except Exception:
            predicted_ns = 0
        _cache[which] = (nc, predicted_ns)
    return _cache[which]


def _run(which, in_maps):
    nc, _ = _get_module(which)
    res = run_bass_kernel_spmd(nc, in_maps, core_ids=list(range(N_CORES)))
    return res.results


def _host_merge(cand_val, cand_idx, zsum, smsum):
    """Exact global top-64 from per-block top-8 candidates; returns
    (top_idx [B,64] int64, top_w [B,64] f32, flags [B] bool)."""
    Bq = cand_val.shape[0]
    base = (np.arange(NB, dtype=np.int64) * BLK).repeat(8)[None, :]   # [1, 1024]
    gidx = cand_idx.astype(np.int64) + base                            # [B, 1024]
    # sort by (-value, index): replicate jax.lax.top_k tie semantics
    ordv = np.argsort(-cand_val, axis=1, kind="stable")
    # stable argsort on -val keeps original (block-order = index-order within
    # equal values ONLY if candidate positions are index-sorted; positions are
    # block-major so equal values tie-break by block position == index order
    # across blocks, but within a block top-8 are value-sorted. For exact-tie
    # values inside one block max_index may even return duplicate positions;
    # rows where the winning set has duplicate or tied values are flagged and
    # recomputed exactly on the host.
    rows = np.arange(Bq)[:, None]
    sidx = gidx[rows, ordv][:, :TOP_K]
    sval = cand_val[rows, ordv][:, :TOP_K]

    # weights, reference formula: attn/(S + 1e-8) with attn = E/Z, S = SM/Z
    denom = (smsum + np.float32(1e-8) * zsum).astype(np.float32)      # [B,1]
    topw = (sval / denom).astype(np.float32)

    # --- safety flags ---
    v64 = sval[:, -1:]                                                 # [B,1]
    blk8 = cand_val.reshape(Bq, NB, 8)
    # a block whose 8th-best is >= the global 64th might hide a 9th member
    overflow = (blk8[:, :, 7] >= v64).any(axis=1)
    # duplicate winner indices (exact-tie artifacts)
    ss = np.sort(sidx, axis=1)
    dup = (ss[:, 1:] == ss[:, :-1]).any(axis=1)
    # winner below threshold (would be masked in reference)
    below = (sval <= (THRESHOLD * zsum).astype(np.float32)).any(axis=1)
    flags = overflow | dup | below
    return sidx, topw, flags


def _host_exact_row(q_row, memory_keys):
    """Full-precision fp32 reference for one row (fallback path)."""
    l = (q_row[None, :] @ memory_keys.T).astype(np.float32)[0] * np.float32(SCALE)
    l = l - l.max()
    e = np.exp(l, dtype=np.float32)
    a = e / e.sum(dtype=np.float32)
    mask = a > np.float32(THRESHOLD)
    am = np.where(mask, a, np.float32(0))
    am = am / (am.sum(dtype=np.float32) + np.float32(1e-8))
    order = np.lexsort((np.arange(M), -am))[:TOP_K]
    return order.astype(np.int64), am[order].astype(np.float32)


def kernel(query, memory_keys, memory_values, top_k=TOP_K, **_ignored):
    assert int(top_k) == TOP_K
    query = np.asarray(query, dtype=np.float32)
    memory_keys = np.asarray(memory_keys, dtype=np.float32)
    memory_values = np.asarray(memory_values, dtype=np.float32)

    # ---------------- Launch A ----------------
    qt_full = np.ascontiguousarray(query.T)          # [D, B]
    kt_full = np.ascontiguousarray(memory_keys.T)    # [D, M]
    if MM_MODE == "fp32":
        in_maps = [
            {"qt": np.ascontiguousarray(qt_full[:, c * NQ:(c + 1) * NQ]), "kt": kt_full}
            for c in range(N_CORES)
        ]
    else:
        qth = qt_full.astype(np.float16)
        qtl = (qt_full - qth.astype(np.float32)).astype(np.float16)
        kth = kt_full.astype(np.float16)
        ktl = (kt_full - kth.astype(np.float32)).astype(np.float16)
        in_maps = [
            {
                "qth": np.ascontiguousarray(qth[:, c * NQ:(c + 1) * NQ]),
                "qtl": np.ascontiguousarray(qtl[:, c * NQ:(c + 1) * NQ]),
                "kth": kth,
                "ktl": ktl,
            }
            for c in range(N_CORES)
        ]
    res_a = _run("a", in_maps)

    cand_val = np.concatenate([r["cand_val"] for r in res_a], axis=0)   # [B, 1024]
    cand_idx = np.concatenate([r["cand_idx"] for r in res_a], axis=0)
    zsum = np.concatenate([r["zsum"] for r in res_a], axis=0)           # [B, 1]
    smsum = np.concatenate([r["smsum"] for r in res_a], axis=0)

    top_idx, top_w, flags = _host_merge(cand_val, cand_idx, zsum, smsum)
    for r in np.nonzero(flags)[0]:
        top_idx[r], top_w[r] = _host_exact_row(query[r], memory_keys)

    # ---------------- Launch B (deduplicated gather) ----------------
    flat_idx = top_idx.reshape(-1)                   # [B*64]
    uniq, inverse = np.unique(flat_idx, return_inverse=True)
    shard_of_u = uniq // MSH
    in_maps_b = []
    sel_lists = []
    for s in range(N_CORES):
        sel = np.nonzero(shard_of_u == s)[0]
        if sel.size > GCAP:
            sel = sel[:GCAP]  # excess unique rows fall back to the host fill
        loc = np.zeros((GCAP, 1), dtype=np.int32)
        loc[:sel.size, 0] = (uniq[sel] - s * MSH).astype(np.int32)
        sel_lists.append(sel)
        in_maps_b.append({
            "ksh": memory_keys[s * MSH:(s + 1) * MSH],
            "vsh": memory_values[s * MSH:(s + 1) * MSH],
            "idx": loc,
        })
    res_b = _run("b", in_maps_b)

    ktab = np.empty((uniq.size, D), dtype=np.float32)
    vtab = np.empty((uniq.size, D), dtype=np.float32)
    covered = np.zeros(uniq.size, dtype=bool)
    for s in range(N_CORES):
        sel = sel_lists[s]
        ktab[sel] = res_b[s]["kg"][:sel.size]
        vtab[sel] = res_b[s]["vg"][:sel.size]
        covered[sel] = True
    if not covered.all():
        rest = np.nonzero(~covered)[0]
        ktab[rest] = memory_keys[uniq[rest]]
        vtab[rest] = memory_values[uniq[rest]]
    retrieved_keys = ktab[inverse].reshape(B, TOP_K, D)
    retrieved_values = vtab[inverse].reshape(B, TOP_K, D)

    global LAST_HW_NS
    LAST_HW_NS = _get_module("a")[1] + _get_module("b")[1]
    return retrieved_keys, retrieved_values, top_w
